# revision 20
# baseline (speedup 1.0000x reference)
"""ChannelAttention3D Trainium2 kernel.

Reference computation (torch-style Conv3d, zero padding):
    avg = mean(x, axis=C); mx = max(x, axis=C)          # x: [B=2, C=32, D=64, H=128, W=128]
    out = sigmoid(conv3d(concat([avg, mx]), W))          # W: [1, 2, 7, 7, 7], pad 3

Strategy (8 NeuronCores):
  - Shard: core k handles batch b=k//4, output d-slices [16*(k%4), 16*(k%4)+16).
    Each core receives its 22 source slices (16 + 3 halo each side, zero padded
    at the volume boundary) as a pre-sliced input tensor.
  - On-chip: ScalarE casts each raw slice fp32->fp16 (same 11-bit significand
    as the PE's fp32r mode for these magnitudes, so this loses little vs an
    fp32r pipeline); DVE then does the channel sum + max as fp16 pairwise
    trees in its 2x packed mode. The 1/32 for the mean is folded into the
    conv weights.
  - Conv: decomposed into 98 H-axis Toeplitz matmuls per output group of 4
    d-slices (one per (channel, kd, kw) tap combination, contraction over
    source h'; fp16 operands, 1 PE cycle/row, fp32 PSUM accumulation).
    Toeplitz matrices are built host-side from W.
  - Sigmoid on ScalarE straight out of PSUM; weight load and output stores go
    through ACT's HWDGE ring so they never stall the input-load FIFO.
"""

import numpy as np

import concourse.bass as bass
import concourse.tile as tile
from concourse import mybir
from concourse.bass_utils import run_bass_kernel_spmd

B, C, D, H, W_DIM = 2, 32, 64, 128, 128
KK, PAD = 7, 3
WH = W_DIM + 2 * PAD               # 134: xc width incl. conv w-halo
NCORES = 8
CORES_PER_B = NCORES // B          # 4
D_PER = D // CORES_PER_B           # 16 output slices per core
SRC = D_PER + 2 * PAD              # 22 source slices per core
NTAPS = 2 * KK * KK                # 98 (c, kd, kw) combos
GSIZE = 4                          # d-slices per output group (fp32 PSUM: N<=512)
GROUPS = D_PER // GSIZE


def _split_multi_waits(nc):
    """The walrus build in this container accepts at most ONE sync-wait per
    instruction. Move surplus waits onto fresh single-wait NoOps placed just
    before the owning instruction (same engine, so sequencer order preserves
    the blocking semantics)."""
    for f in nc.m.functions:
        for bb in f.blocks:
            new_list = []
            changed = False
            for ins in bb.instructions:
                si = ins.sync_info
                waits = list(si.on_wait) if si is not None else []
                if len(waits) > 1:
                    changed = True
                    for i, w in enumerate(waits[:-1]):
                        nop = mybir.InstNoOp(name=f"{ins.name}_ws{i}",
                                             ins=[], outs=[])
                        nop.engine = ins.engine
                        nop.sync_info = mybir.SyncInfo(on_wait=[w],
                                                       on_update=[])
                        new_list.append(nop)
                    ins.sync_info = mybir.SyncInfo(on_wait=[waits[-1]],
                                                   on_update=list(si.on_update))
                new_list.append(ins)
            if changed:
                bb.instructions[:] = new_list


def _taps():
    out = []
    for kd in range(KK):
        for c in range(2):
            for kw in range(KK):
                out.append((c, kd, kw))
    return out


def _build_toeplitz(w: np.ndarray) -> np.ndarray:
    """lhsT stack [NTAPS, 128, 128]: T[ti][h', h] = Wc[c, kd, h'-h+3, kw]."""
    wc = w[0].astype(np.float64).copy()     # [2, 7, 7, 7]
    wc[0] /= C                              # fold channel-mean scale into taps
    lhsT = np.zeros((NTAPS, H, H), np.float32)
    hs = np.arange(H)
    for ti, (c, kd, kw) in enumerate(_taps()):
        for dh in range(KK):
            hp = hs + dh - PAD
            m = (hp >= 0) & (hp < H)
            lhsT[ti, hp[m], hs[m]] = wc[c, kd, dh, kw]
    return lhsT.astype(np.float16)


def _build_program():
    nc = bass.Bass()
    xs_d = nc.declare_dram_parameter("xs", [C, SRC, H, W_DIM], mybir.dt.float32,
                                     isOutput=False)
    tw_d = nc.declare_dram_parameter("tw", [NTAPS, H, H], mybir.dt.float16,
                                     isOutput=False)
    y_d = nc.declare_dram_parameter("y", [D_PER, H, W_DIM], mybir.dt.float32,
                                    isOutput=True)

    with tile.TileContext(nc) as tc:
        with (
            tc.tile_pool(name="xs", bufs=4) as xs_pool,
            tc.tile_pool(name="xh", bufs=4) as xh_pool,
            tc.tile_pool(name="xc", bufs=1) as xc_pool,
            tc.tile_pool(name="tw", bufs=1) as tw_pool,
            tc.tile_pool(name="out", bufs=4) as out_pool,
            tc.tile_pool(name="psum", bufs=4, space="PSUM") as psum_pool,
            tc.tile_pool(name="scr", bufs=1) as scr_pool,
        ):
            tw_sb = tw_pool.tile([H, NTAPS, H], mybir.dt.float16)
            # SWDGE queue: keeps the big weight load off both HWDGE rings
            # (SP carries the input loads, ACT carries the output stores).
            nc.gpsimd.dma_start(out=tw_sb, in_=tw_d.rearrange("t hp hm -> hp t hm"))

            xc_sum = xc_pool.tile([H, SRC, WH], mybir.dt.float16, tag="xc_sum")
            xc_max = xc_pool.tile([H, SRC, WH], mybir.dt.float16, tag="xc_max")
            xc = [xc_sum, xc_max]
            for t in xc:
                nc.gpsimd.memset(t[:], 0.0)   # conv w-halo zeros

            def dve_tree(xth, s, c_idx, op):
                """32-way channel reduction as an fp16 pairwise tree on DVE
                (2x packed mode), ping-ponging through scratch columns
                [0:16][16:24][24:28][28:30]; the final pair lands in xc."""
                scr = scr_pool.tile([H, 30, W_DIM], mybir.dt.float16,
                                    name="scr_t", tag=f"scr{c_idx}")
                src = xth.rearrange("h (a b) w -> h a b w", b=2)
                with nc.allow_low_precision(reason="fp16 channel tree"):
                    nc.vector.tensor_tensor(out=scr[:, 0:16], in0=src[:, :, 0],
                                            in1=src[:, :, 1], op=op)
                    base = 0
                    for n in (8, 4, 2):
                        prev = scr[:, base:base + 2 * n].rearrange(
                            "h (a b) w -> h a b w", b=2)
                        nxt = base + 2 * n
                        nc.vector.tensor_tensor(out=scr[:, nxt:nxt + n],
                                                in0=prev[:, :, 0],
                                                in1=prev[:, :, 1], op=op)
                        base = nxt
                    nc.vector.tensor_tensor(out=xc[c_idx][:, s, PAD:PAD + W_DIM],
                                            in0=scr[:, 28], in1=scr[:, 29],
                                            op=op)

            psum_tiles = {}
            # Interleave: right after slice s is reduced, emit every matmul
            # whose last needed source slice is s, so PE overlaps with the
            # reduction pipeline instead of waiting for all of it.
            for s in range(SRC):
                xt = xs_pool.tile([H, C, W_DIM], mybir.dt.float32)
                nc.sync.dma_start(out=xt, in_=xs_d[:, s].rearrange("c h w -> h c w"))
                xth = xh_pool.tile([H, C, W_DIM], mybir.dt.float16, name="xth")
                # fp32->fp16 cast: mostly on ScalarE; every 4th slice on DVE
                # (2x single-src mode) — balances the two engines' spans.
                if s % 4 == 3:
                    nc.vector.tensor_copy(out=xth, in_=xt)
                else:
                    nc.scalar.copy(out=xth, in_=xt)
                dve_tree(xth, s, 0, mybir.AluOpType.add)
                dve_tree(xth, s, 1, mybir.AluOpType.max)

                for g in range(GROUPS):
                    kd = s - GSIZE * g - (GSIZE - 1)  # last-needed slice
                    if not (0 <= kd < KK):
                        continue
                    if kd == 0:
                        psum_tiles[g] = psum_pool.tile([H, GSIZE, W_DIM],
                                                       mybir.dt.float32,
                                                       name="psum_g", tag="psum")
                    lo = GSIZE * g + kd
                    for c in range(2):
                        for kw in range(KK):
                            ti = (kd * 2 + c) * KK + kw
                            nc.tensor.matmul(
                                psum_tiles[g],
                                tw_sb[:, ti, :],
                                xc[c][:, lo:lo + GSIZE, kw:kw + W_DIM],
                                start=(ti == 0),
                                stop=(ti == NTAPS - 1),
                            )
                    if kd == KK - 1:
                        out_t = out_pool.tile([H, GSIZE, W_DIM], mybir.dt.float32)
                        nc.scalar.activation(out_t, psum_tiles.pop(g),
                                             mybir.ActivationFunctionType.Sigmoid)
                        nc.scalar.dma_start(
                            out=y_d[GSIZE * g:GSIZE * (g + 1)].rearrange(
                                "d h w -> h d w"),
                            in_=out_t)
    return nc


_program_cache = None


def kernel(x: np.ndarray, W: np.ndarray) -> np.ndarray:
    global _program_cache

    x = np.ascontiguousarray(x, dtype=np.float32)
    lhsT = _build_toeplitz(np.asarray(W, dtype=np.float32))

    in_maps = []
    for k in range(NCORES):
        b, q = divmod(k, CORES_PER_B)
        dlo = q * D_PER - PAD
        dhi = dlo + SRC
        lo_c, hi_c = max(dlo, 0), min(dhi, D)
        xs = np.zeros((C, SRC, H, W_DIM), np.float32)
        xs[:, lo_c - dlo:hi_c - dlo] = x[b, :, lo_c:hi_c]
        in_maps.append({"xs": xs, "tw": lhsT})

    if _program_cache is None:
        nc = _build_program()
        _split_multi_waits(nc)
        _program_cache = nc
    nc = _program_cache

    res = run_bass_kernel_spmd(nc, in_maps, core_ids=list(range(NCORES)))

    out = np.empty((B, 1, D, H, W_DIM), np.float32)
    for k in range(NCORES):
        b, q = divmod(k, CORES_PER_B)
        out[b, 0, q * D_PER:(q + 1) * D_PER] = res.results[k]["y"]
    return out
